# revision 11
# baseline (speedup 1.0000x reference)
"""Trainium2 Bass kernel for nn_Displacement (displacement-gate matrix apply).

Problem: r, phi: (32,) f32; state: (32, 32, 32, 32, 32) f32.
  D[b] = displacement-gate matrix (32x32 complex64),
  out = einsum('bmn,bnxyz->bmxyz', D, state)  -> complex64 (32,32,32,32,32).

Strategy: pure data-parallel over the batch axis across 8 NeuronCores
(4 batches/core).  Per core:
  - Compute the 4 D matrices on-device via the numerically stable closed
    form  D[m,n] = e^{-r^2/2} sqrt(n!/m!) alpha^{m-n} L_n^{(m-n)}(r^2)
    (m>=n; mirrored with beta=-conj(alpha) for m<n), using the Laguerre
    three-term recurrence.  (The reference's own f32 column recurrence is
    catastrophically unstable for |r|>2 -- this form is accurate to ~1e-5
    of float64 truth in f32.)
  - Store D^T per batch into a block-diagonal 128x128 fp32 lhsT (real and
    imag parts separately).
  - Stream state (128, 32768) f32 through the tensor engine in 512-column
    chunks: one fp32 matmul per part (re/im) per chunk against the
    block-diagonal lhsT (contraction dim = 128 = 4 batches x 32).
  - Interleave re/im into complex64 layout on-chip (strided copies from
    PSUM into SBUF) and DMA out 2MB at a time.

The kernel is memory-bound: 16.8 MB in + 33.6 MB out per core (~140us at
~360 GB/s per-core HBM bandwidth).
"""

import math

import numpy as np

B = 32          # batch
C = 32          # cutoff
NCORES = 8
BPC = B // NCORES          # batches per core = 4
NFREE = C * C * C          # 32768 state columns per (b, n)
P = 128                    # partitions

# ---- constant-table layout (per-partition broadcast, (BPC, TABW) f32) ----
# Laguerre steps n -> n+1 for n = 1..30:
#   c1s[n][k] = (2n+k+1)/(n+1),  c2s[n][k] = (n+k)/(n+1)        k = 0..31
# row-1 init:   kp1[k] = k+1
# x scalings:   invn1[n-1] = 1/(n+1)  for n = 1..30
# factor diagonals d = 0..31:  fd[d][j] = sqrt(j!/(j+d)!)  j = 0..31-d
_C1S_OFF = 0                      # 30*32 = 960
_C2S_OFF = 960                    # 30*32 = 960
_KP1_OFF = 1920                   # 32
_INVN1_OFF = 1952                 # 30
_FD_OFF = 1984                    # sum_{d=0}^{31} (32-d) = 528
TABW = 2560                       # padded

_FD_OFFS = [0] * C
_off = _FD_OFF
for _d in range(C):
    _FD_OFFS[_d] = _off
    _off += C - _d
assert _off <= TABW


def _build_tables() -> np.ndarray:
    lg = [math.lgamma(k + 1) for k in range(C)]
    tab = np.zeros(TABW, dtype=np.float64)
    for n in range(1, C - 1):
        for k in range(C):
            tab[_C1S_OFF + (n - 1) * C + k] = (2 * n + k + 1) / (n + 1)
            tab[_C2S_OFF + (n - 1) * C + k] = (n + k) / (n + 1)
    for k in range(C):
        tab[_KP1_OFF + k] = k + 1
    for n in range(1, C - 1):
        tab[_INVN1_OFF + (n - 1)] = 1.0 / (n + 1)
    for d in range(C):
        for j in range(C - d):
            tab[_FD_OFFS[d] + j] = math.exp(0.5 * (lg[j] - lg[j + d]))
    tab = tab.astype(np.float32)
    return np.broadcast_to(tab, (BPC, TABW)).copy()


def build_nc(niter: int = 1, use_f32r: bool = True, out_on_act: bool = False,
             chunk_f32: int = 4096, st_bufs: int = 5, ob_bufs: int = 2):
    """Build + compile the per-core Bass module (SPMD, same NEFF all cores).

    niter > 1 repeats the main (matmul + store) loop for steady-state HW
    timing; the D computation runs once.

    use_f32r: run the tensor-engine matmuls in float32r mode (4x faster
    streaming than plain fp32 on TRN2).
    """
    import concourse.mybir as mybir
    from concourse import bacc
    from concourse.alu_op_type import AluOpType as OP
    from concourse.mybir import ActivationFunctionType as AF
    from concourse.tile import TileContext

    F32 = mybir.dt.float32
    MMDT = mybir.dt.float32r if use_f32r else mybir.dt.float32
    nc = bacc.Bacc("TRN2", target_bir_lowering=False)

    state = nc.dram_tensor("state", [P, NFREE], F32, kind="ExternalInput")
    rphi = nc.dram_tensor("rphi", [BPC, 2], F32, kind="ExternalInput")
    tab = nc.dram_tensor("tab", [BPC, TABW], F32, kind="ExternalInput")
    out = nc.dram_tensor("out", [P, 2 * NFREE], F32, kind="ExternalOutput")

    with TileContext(nc) as tc:
        with (
            tc.tile_pool(name="const", bufs=1) as cpool,
            tc.tile_pool(name="scratch", bufs=4) as spool,
            tc.tile_pool(name="stpool", bufs=st_bufs) as stpool,
            tc.tile_pool(name="opool", bufs=ob_bufs) as opool,
            tc.tile_pool(name="psum", bufs=8, space="PSUM") as ppool,
        ):
            rphi_t = cpool.tile([BPC, 2], F32)
            nc.sync.dma_start(out=rphi_t[:], in_=rphi[:])
            tab_t = cpool.tile([BPC, TABW], F32)
            nc.sync.dma_start(out=tab_t[:], in_=tab[:])

            lhsT_re = cpool.tile([P, P], MMDT)
            lhsT_im = cpool.tile([P, P], MMDT)

            Dt_re = cpool.tile([BPC, C * C], F32)   # Dt[b, n*32+m] = D_b[m,n]
            Dt_im = cpool.tile([BPC, C * C], F32)

            r = rphi_t[:, 0:1]
            ph = rphi_t[:, 1:2]

            # ---- per-batch scalars ----
            # ACT's scale/bias path is low-precision (cos via Sin(x+pi/2)
            # measured 4.5e-4 abs err); raw Sin is ~5e-8.  So: sin directly,
            # cos = 1 - 2 sin^2(phi/2) with the halving done exactly on DVE.
            sphi = cpool.tile([BPC, 1], F32)
            nc.scalar.activation(sphi[:], ph, AF.Sin)
            phh = cpool.tile([BPC, 1], F32)
            nc.vector.tensor_scalar_mul(phh[:], ph, 0.5)
            shh = cpool.tile([BPC, 1], F32)
            nc.scalar.activation(shh[:], phh[:], AF.Sin)
            sh2 = cpool.tile([BPC, 1], F32)
            nc.vector.tensor_tensor(sh2[:], shh[:], shh[:], OP.mult)
            cphi = cpool.tile([BPC, 1], F32)
            nc.vector.tensor_scalar(cphi[:], sh2[:], -2.0, 1.0, OP.mult, OP.add)
            a0re = cpool.tile([BPC, 1], F32)   # alpha = r e^{i phi}
            nc.vector.tensor_tensor(a0re[:], r, cphi[:], OP.mult)
            a0im = cpool.tile([BPC, 1], F32)
            nc.vector.tensor_tensor(a0im[:], r, sphi[:], OP.mult)
            x2 = cpool.tile([BPC, 1], F32)     # x = r^2
            nc.vector.tensor_tensor(x2[:], r, r, OP.mult)
            xh = cpool.tile([BPC, 1], F32)     # -x/2 (exact, on DVE)
            nc.vector.tensor_scalar_mul(xh[:], x2[:], -0.5)
            e2 = cpool.tile([BPC, 1], F32)     # exp(-x/2)
            nc.scalar.activation(e2[:], xh[:], AF.Exp)
            b0re = cpool.tile([BPC, 1], F32)   # beta = -conj(alpha)
            nc.vector.tensor_scalar_mul(b0re[:], a0re[:], -1.0)
            b0im = a0im                         # beta_im = alpha_im

            # ---- powers alpha^d, beta^d (d = 0..31) by repeated doubling ----
            def powers(c_re, c_im, negate_re):
                Pre = cpool.tile([BPC, C], F32, tag=f"Pre{negate_re}")
                Pim = cpool.tile([BPC, C], F32, tag=f"Pim{negate_re}")
                nc.vector.memset(Pre[:, 0:1], 1.0)
                nc.vector.memset(Pim[:, 0:1], 0.0)
                nc.vector.tensor_copy(Pre[:, 1:2], c_re[:])
                nc.vector.tensor_copy(Pim[:, 1:2], c_im[:])
                cre, cim = c_re, c_im
                for s in range(1, 5):
                    L = 1 << s
                    t1 = spool.tile([BPC, 1], F32, tag="sq1")
                    nc.vector.tensor_tensor(t1[:], cre[:], cre[:], OP.mult)
                    t2 = spool.tile([BPC, 1], F32, tag="sq2")
                    nc.vector.tensor_tensor(t2[:], cim[:], cim[:], OP.mult)
                    c2re = cpool.tile([BPC, 1], F32, tag=f"c2re{negate_re}{s}")
                    nc.vector.tensor_tensor(c2re[:], t1[:], t2[:], OP.subtract)
                    t3 = spool.tile([BPC, 1], F32, tag="sq3")
                    nc.vector.tensor_tensor(t3[:], cre[:], cim[:], OP.mult)
                    c2im = cpool.tile([BPC, 1], F32, tag=f"c2im{negate_re}{s}")
                    nc.vector.tensor_tensor(c2im[:], t3[:], t3[:], OP.add)
                    tre = spool.tile([BPC, C // 2], F32, tag="pw1")
                    nc.vector.tensor_scalar_mul(tre[:, :L], Pim[:, 0:L], c2im[:])
                    nc.vector.scalar_tensor_tensor(
                        Pre[:, L:2 * L], Pre[:, 0:L], c2re[:], tre[:, :L],
                        OP.mult, OP.subtract,
                    )
                    tim = spool.tile([BPC, C // 2], F32, tag="pw2")
                    nc.vector.tensor_scalar_mul(tim[:, :L], Pre[:, 0:L], c2im[:])
                    nc.vector.scalar_tensor_tensor(
                        Pim[:, L:2 * L], Pim[:, 0:L], c2re[:], tim[:, :L],
                        OP.mult, OP.add,
                    )
                    cre, cim = c2re, c2im
                return Pre, Pim

            PaRe, PaIm = powers(a0re, a0im, 0)
            PbRe, PbIm = powers(b0re, b0im, 1)

            # ---- Laguerre table: Lag[b, n*32+k] = L_n^{(k)}(x), n+k < 32 ----
            Lag = cpool.tile([BPC, C * C], F32)
            nc.vector.memset(Lag[:, 0:C], 1.0)                       # row 0
            nc.vector.tensor_scalar(
                Lag[:, C:2 * C], tab_t[:, _KP1_OFF:_KP1_OFF + C],
                x2[:], None, OP.subtract,
            )                                                         # row 1
            # xs[n-1] = x/(n+1) for n = 1..30
            xs = cpool.tile([BPC, C - 2], F32)
            nc.vector.tensor_scalar_mul(
                xs[:], tab_t[:, _INVN1_OFF:_INVN1_OFF + C - 2], x2[:]
            )
            for n in range(1, C - 1):
                prev = Lag[:, (n - 1) * C: n * C]
                cur = Lag[:, n * C: (n + 1) * C]
                nxt = Lag[:, (n + 1) * C: (n + 2) * C]
                c1v = tab_t[:, _C1S_OFF + (n - 1) * C: _C1S_OFF + n * C]
                c2v = tab_t[:, _C2S_OFF + (n - 1) * C: _C2S_OFF + n * C]
                t1 = spool.tile([BPC, C], F32, tag="lag1")
                nc.vector.tensor_scalar(t1[:], c1v, xs[:, n - 1: n], None,
                                        OP.subtract)
                t2 = spool.tile([BPC, C], F32, tag="lag2")
                nc.vector.tensor_tensor(t2[:], t1[:], cur, OP.mult)
                t3 = spool.tile([BPC, C], F32, tag="lag3")
                nc.vector.tensor_tensor(t3[:], c2v, prev, OP.mult)
                nc.vector.tensor_tensor(nxt, t2[:], t3[:], OP.subtract)

            # ---- assemble Dt[b, n*32+m] = D_b[m, n] (without e^{-x/2}) ----
            # upper part m = n+d (d >= 0): diag positions n*33 + d, n = 0..31-d
            #   value = fd[d][n] * Lag[n, d] * alpha^d
            # lower part n = m+d (d >= 1): diag positions m*33 + 32*d, m = 0..31-d
            #   value = fd[d][m] * Lag[m, d] * beta^d
            for d in range(C):
                w = C - d
                lagv = Lag[:, d: d + (w - 1) * C + 1: C]          # (BPC, w)
                fdv = tab_t[:, _FD_OFFS[d]: _FD_OFFS[d] + w]
                t = spool.tile([BPC, C], F32, tag="asm")
                nc.vector.tensor_tensor(t[:, :w], lagv, fdv, OP.mult)
                up = slice(d, d + (w - 1) * 33 + 1, 33)
                nc.vector.tensor_scalar_mul(
                    Dt_re[:, up], t[:, :w], PaRe[:, d: d + 1])
                nc.vector.tensor_scalar_mul(
                    Dt_im[:, up], t[:, :w], PaIm[:, d: d + 1])
                if d >= 1:
                    lo = slice(32 * d, 32 * d + (w - 1) * 33 + 1, 33)
                    nc.vector.tensor_scalar_mul(
                        Dt_re[:, lo], t[:, :w], PbRe[:, d: d + 1])
                    nc.vector.tensor_scalar_mul(
                        Dt_im[:, lo], t[:, :w], PbIm[:, d: d + 1])
            # fold e^{-x/2}
            nc.vector.tensor_scalar_mul(Dt_re[:], Dt_re[:], e2[:])
            nc.vector.tensor_scalar_mul(Dt_im[:], Dt_im[:], e2[:])

            # ---- scatter D^T blocks onto the block-diagonal lhsT ----
            # lhsT[b*32+n, b*32+m] = D_b[m, n] = Dt[b, n*32+m]
            # (memset can't write f32r; zero via DMA from a zeroed f32 tile)
            zscr = cpool.tile([P, P], F32)
            nc.vector.memset(zscr[:], 0.0)
            nc.sync.dma_start(out=lhsT_re[:], in_=zscr[:].bitcast(MMDT))
            nc.sync.dma_start(out=lhsT_im[:], in_=zscr[:].bitcast(MMDT))
            for b in range(BPC):
                nc.sync.dma_start(
                    out=lhsT_re[C * b: C * (b + 1), C * b: C * (b + 1)],
                    in_=Dt_re[b: b + 1, :].bitcast(MMDT),
                )
                nc.sync.dma_start(
                    out=lhsT_im[C * b: C * (b + 1), C * b: C * (b + 1)],
                    in_=Dt_im[b: b + 1, :].bitcast(MMDT),
                )

            # ---- main loop: 64 chunks of 512 state columns ----
            # state DMA'd in 2MB (128 x 4096 f32) pieces, output written in
            # 2MB pieces (4 chunks x 1024 interleaved f32).
            out_eng = nc.scalar if out_on_act else nc.sync
            n_sub = chunk_f32 // 512          # 512-col matmuls per state piece
            n_grp = NFREE // chunk_f32        # state pieces
            for _ in range(niter):
                for g in range(n_grp):
                    schunk = stpool.tile([P, chunk_f32], MMDT, tag="st")
                    nc.sync.dma_start(
                        out=schunk[:],
                        in_=state[:, g * chunk_f32: (g + 1) * chunk_f32]
                        .bitcast(MMDT),
                    )
                    obuf = opool.tile([P, 2 * chunk_f32], F32, tag="ob")
                    for h in range(n_sub // 4):
                        psr, psi = [], []
                        for q in range(h * 4, h * 4 + 4):
                            ps = ppool.tile([P, 512], F32, tag="ps")
                            nc.tensor.matmul(
                                ps[:], lhsT_re[:],
                                schunk[:, q * 512: (q + 1) * 512],
                                start=True, stop=True,
                            )
                            psr.append(ps)
                        for q in range(h * 4, h * 4 + 4):
                            ps = ppool.tile([P, 512], F32, tag="ps")
                            nc.tensor.matmul(
                                ps[:], lhsT_im[:],
                                schunk[:, q * 512: (q + 1) * 512],
                                start=True, stop=True,
                            )
                            psi.append(ps)
                        for i, q in enumerate(range(h * 4, h * 4 + 4)):
                            nc.vector.tensor_copy(
                                obuf[:, q * 1024: (q + 1) * 1024: 2], psr[i][:]
                            )
                            nc.scalar.copy(
                                obuf[:, q * 1024 + 1: (q + 1) * 1024: 2],
                                psi[i][:]
                            )
                    out_eng.dma_start(
                        out=out[:, g * 2 * chunk_f32: (g + 1) * 2 * chunk_f32],
                        in_=obuf[:],
                    )

    nc.compile()
    return nc


_CACHE: dict = {}


def _get_nc(niter: int = 1):
    if niter not in _CACHE:
        _CACHE[niter] = build_nc(niter)
    return _CACHE[niter]


def kernel(r: np.ndarray, phi: np.ndarray, state: np.ndarray) -> np.ndarray:
    from concourse.bass_utils import run_bass_kernel_spmd

    r = np.ascontiguousarray(r, dtype=np.float32)
    phi = np.ascontiguousarray(phi, dtype=np.float32)
    state = np.ascontiguousarray(state, dtype=np.float32)
    assert r.shape == (B,) and phi.shape == (B,)
    assert state.shape == (B, C, C, C, C)

    nc = _get_nc(1)
    tab = _build_tables()
    in_maps = []
    for c in range(NCORES):
        b0 = c * BPC
        in_maps.append({
            "state": state[b0: b0 + BPC].reshape(P, NFREE),
            "rphi": np.stack(
                [r[b0: b0 + BPC], phi[b0: b0 + BPC]], axis=1
            ).astype(np.float32),
            "tab": tab,
        })

    res = run_bass_kernel_spmd(nc, in_maps, core_ids=list(range(NCORES)))

    outp = np.empty((B, C, C, C, C), dtype=np.complex64)
    for c in range(NCORES):
        b0 = c * BPC
        blk = res.results[c]["out"]
        outp[b0: b0 + BPC] = (
            np.ascontiguousarray(blk).view(np.complex64).reshape(BPC, C, C, C, C)
        )
    return outp


# revision 12
# speedup vs baseline: 1.2503x; 1.2503x over previous
"""Trainium2 Bass kernel for nn_Displacement (displacement-gate matrix apply).

Problem: r, phi: (32,) f32; state: (32, 32, 32, 32, 32) f32.
  D[b] = displacement-gate matrix (32x32 complex64),
  out = einsum('bmn,bnxyz->bmxyz', D, state)  -> complex64 (32,32,32,32,32).

Strategy: pure data-parallel over the batch axis across 8 NeuronCores
(4 batches/core).  Per core:
  - Compute the 4 D matrices on-device via the numerically stable closed
    form  D[m,n] = e^{-r^2/2} sqrt(n!/m!) alpha^{m-n} L_n^{(m-n)}(r^2)
    (m>=n; mirrored with beta=-conj(alpha) for m<n), using the Laguerre
    three-term recurrence.  (The reference's own f32 column recurrence is
    catastrophically unstable for |r|>2 -- this form is accurate to ~1e-5
    of float64 truth in f32.)
  - Store D^T per batch into a block-diagonal 128x128 fp32 lhsT (real and
    imag parts separately).
  - Stream state (128, 32768) f32 through the tensor engine in 512-column
    chunks: one fp32 matmul per part (re/im) per chunk against the
    block-diagonal lhsT (contraction dim = 128 = 4 batches x 32).
  - Interleave re/im into complex64 layout on-chip (strided copies from
    PSUM into SBUF) and DMA out 2MB at a time.

The kernel is memory-bound: 16.8 MB in + 33.6 MB out per core (~140us at
~360 GB/s per-core HBM bandwidth).
"""

import math

import numpy as np

B = 32          # batch
C = 32          # cutoff
NCORES = 8
BPC = B // NCORES          # batches per core = 4
NFREE = C * C * C          # 32768 state columns per (b, n)
P = 128                    # partitions

# ---- constant-table layout (per-partition broadcast, (BPC, TABW) f32) ----
# Laguerre steps n -> n+1 for n = 1..30:
#   c1s[n][k] = (2n+k+1)/(n+1),  c2s[n][k] = (n+k)/(n+1)        k = 0..31
# row-1 init:   kp1[k] = k+1
# x scalings:   invn1[n-1] = 1/(n+1)  for n = 1..30
# factor diagonals d = 0..31:  fd[d][j] = sqrt(j!/(j+d)!)  j = 0..31-d
_C1S_OFF = 0                      # 30*32 = 960
_C2S_OFF = 960                    # 30*32 = 960
_KP1_OFF = 1920                   # 32
_INVN1_OFF = 1952                 # 30
_FD_OFF = 1984                    # sum_{d=0}^{31} (32-d) = 528
TABW = 2560                       # padded

_FD_OFFS = [0] * C
_off = _FD_OFF
for _d in range(C):
    _FD_OFFS[_d] = _off
    _off += C - _d
assert _off <= TABW


def _build_tables() -> np.ndarray:
    lg = [math.lgamma(k + 1) for k in range(C)]
    tab = np.zeros(TABW, dtype=np.float64)
    for n in range(1, C - 1):
        for k in range(C):
            tab[_C1S_OFF + (n - 1) * C + k] = (2 * n + k + 1) / (n + 1)
            tab[_C2S_OFF + (n - 1) * C + k] = (n + k) / (n + 1)
    for k in range(C):
        tab[_KP1_OFF + k] = k + 1
    for n in range(1, C - 1):
        tab[_INVN1_OFF + (n - 1)] = 1.0 / (n + 1)
    for d in range(C):
        for j in range(C - d):
            tab[_FD_OFFS[d] + j] = math.exp(0.5 * (lg[j] - lg[j + d]))
    tab = tab.astype(np.float32)
    return np.broadcast_to(tab, (BPC, TABW)).copy()


def build_nc(niter: int = 1, use_f32r: bool = True, out_on_act: bool = False,
             st_f32: int = 4096, ob_f32: int = 4096,
             st_bufs: int = 6, ob_bufs: int = 3):
    """Build + compile the per-core Bass module (SPMD, same NEFF all cores).

    niter > 1 repeats the main (matmul + store) loop for steady-state HW
    timing; the D computation runs once.

    use_f32r: run the tensor-engine matmuls in float32r mode (4x faster
    streaming than plain fp32 on TRN2).
    """
    import concourse.mybir as mybir
    from concourse import bacc
    from concourse.alu_op_type import AluOpType as OP
    from concourse.mybir import ActivationFunctionType as AF
    from concourse.tile import TileContext

    F32 = mybir.dt.float32
    MMDT = mybir.dt.float32r if use_f32r else mybir.dt.float32
    nc = bacc.Bacc("TRN2", target_bir_lowering=False)

    state = nc.dram_tensor("state", [P, NFREE], F32, kind="ExternalInput")
    rphi = nc.dram_tensor("rphi", [BPC, 2], F32, kind="ExternalInput")
    tab = nc.dram_tensor("tab", [BPC, TABW], F32, kind="ExternalInput")
    out = nc.dram_tensor("out", [P, 2 * NFREE], F32, kind="ExternalOutput")

    with TileContext(nc) as tc:
        with (
            tc.tile_pool(name="const", bufs=1) as cpool,
            tc.tile_pool(name="scratch", bufs=4) as spool,
            tc.tile_pool(name="stpool", bufs=st_bufs) as stpool,
            tc.tile_pool(name="opool", bufs=ob_bufs) as opool,
            tc.tile_pool(name="psum", bufs=8, space="PSUM") as ppool,
        ):
            rphi_t = cpool.tile([BPC, 2], F32)
            nc.sync.dma_start(out=rphi_t[:], in_=rphi[:])
            tab_t = cpool.tile([BPC, TABW], F32)
            nc.sync.dma_start(out=tab_t[:], in_=tab[:])

            lhsT_re = cpool.tile([P, P], MMDT)
            lhsT_im = cpool.tile([P, P], MMDT)

            Dt_re = cpool.tile([BPC, C * C], F32)   # Dt[b, n*32+m] = D_b[m,n]
            Dt_im = cpool.tile([BPC, C * C], F32)

            r = rphi_t[:, 0:1]
            ph = rphi_t[:, 1:2]

            # ---- per-batch scalars ----
            # ACT's scale/bias path is low-precision (cos via Sin(x+pi/2)
            # measured 4.5e-4 abs err); raw Sin is ~5e-8.  So: sin directly,
            # cos = 1 - 2 sin^2(phi/2) with the halving done exactly on DVE.
            sphi = cpool.tile([BPC, 1], F32)
            nc.scalar.activation(sphi[:], ph, AF.Sin)
            phh = cpool.tile([BPC, 1], F32)
            nc.vector.tensor_scalar_mul(phh[:], ph, 0.5)
            shh = cpool.tile([BPC, 1], F32)
            nc.scalar.activation(shh[:], phh[:], AF.Sin)
            sh2 = cpool.tile([BPC, 1], F32)
            nc.vector.tensor_tensor(sh2[:], shh[:], shh[:], OP.mult)
            cphi = cpool.tile([BPC, 1], F32)
            nc.vector.tensor_scalar(cphi[:], sh2[:], -2.0, 1.0, OP.mult, OP.add)
            a0re = cpool.tile([BPC, 1], F32)   # alpha = r e^{i phi}
            nc.vector.tensor_tensor(a0re[:], r, cphi[:], OP.mult)
            a0im = cpool.tile([BPC, 1], F32)
            nc.vector.tensor_tensor(a0im[:], r, sphi[:], OP.mult)
            x2 = cpool.tile([BPC, 1], F32)     # x = r^2
            nc.vector.tensor_tensor(x2[:], r, r, OP.mult)
            xh = cpool.tile([BPC, 1], F32)     # -x/2 (exact, on DVE)
            nc.vector.tensor_scalar_mul(xh[:], x2[:], -0.5)
            e2 = cpool.tile([BPC, 1], F32)     # exp(-x/2)
            nc.scalar.activation(e2[:], xh[:], AF.Exp)
            b0re = cpool.tile([BPC, 1], F32)   # beta = -conj(alpha)
            nc.vector.tensor_scalar_mul(b0re[:], a0re[:], -1.0)
            b0im = a0im                         # beta_im = alpha_im

            # ---- powers alpha^d, beta^d (d = 0..31) by repeated doubling ----
            def powers(c_re, c_im, negate_re):
                Pre = cpool.tile([BPC, C], F32, tag=f"Pre{negate_re}")
                Pim = cpool.tile([BPC, C], F32, tag=f"Pim{negate_re}")
                nc.vector.memset(Pre[:, 0:1], 1.0)
                nc.vector.memset(Pim[:, 0:1], 0.0)
                nc.vector.tensor_copy(Pre[:, 1:2], c_re[:])
                nc.vector.tensor_copy(Pim[:, 1:2], c_im[:])
                cre, cim = c_re, c_im
                for s in range(1, 5):
                    L = 1 << s
                    t1 = spool.tile([BPC, 1], F32, tag="sq1")
                    nc.vector.tensor_tensor(t1[:], cre[:], cre[:], OP.mult)
                    t2 = spool.tile([BPC, 1], F32, tag="sq2")
                    nc.vector.tensor_tensor(t2[:], cim[:], cim[:], OP.mult)
                    c2re = cpool.tile([BPC, 1], F32, tag=f"c2re{negate_re}{s}")
                    nc.vector.tensor_tensor(c2re[:], t1[:], t2[:], OP.subtract)
                    t3 = spool.tile([BPC, 1], F32, tag="sq3")
                    nc.vector.tensor_tensor(t3[:], cre[:], cim[:], OP.mult)
                    c2im = cpool.tile([BPC, 1], F32, tag=f"c2im{negate_re}{s}")
                    nc.vector.tensor_tensor(c2im[:], t3[:], t3[:], OP.add)
                    tre = spool.tile([BPC, C // 2], F32, tag="pw1")
                    nc.vector.tensor_scalar_mul(tre[:, :L], Pim[:, 0:L], c2im[:])
                    nc.vector.scalar_tensor_tensor(
                        Pre[:, L:2 * L], Pre[:, 0:L], c2re[:], tre[:, :L],
                        OP.mult, OP.subtract,
                    )
                    tim = spool.tile([BPC, C // 2], F32, tag="pw2")
                    nc.vector.tensor_scalar_mul(tim[:, :L], Pre[:, 0:L], c2im[:])
                    nc.vector.scalar_tensor_tensor(
                        Pim[:, L:2 * L], Pim[:, 0:L], c2re[:], tim[:, :L],
                        OP.mult, OP.add,
                    )
                    cre, cim = c2re, c2im
                return Pre, Pim

            PaRe, PaIm = powers(a0re, a0im, 0)
            PbRe, PbIm = powers(b0re, b0im, 1)

            # ---- Laguerre table: Lag[b, n*32+k] = L_n^{(k)}(x), n+k < 32 ----
            Lag = cpool.tile([BPC, C * C], F32)
            nc.vector.memset(Lag[:, 0:C], 1.0)                       # row 0
            nc.vector.tensor_scalar(
                Lag[:, C:2 * C], tab_t[:, _KP1_OFF:_KP1_OFF + C],
                x2[:], None, OP.subtract,
            )                                                         # row 1
            # xs[n-1] = x/(n+1) for n = 1..30
            xs = cpool.tile([BPC, C - 2], F32)
            nc.vector.tensor_scalar_mul(
                xs[:], tab_t[:, _INVN1_OFF:_INVN1_OFF + C - 2], x2[:]
            )
            for n in range(1, C - 1):
                prev = Lag[:, (n - 1) * C: n * C]
                cur = Lag[:, n * C: (n + 1) * C]
                nxt = Lag[:, (n + 1) * C: (n + 2) * C]
                c1v = tab_t[:, _C1S_OFF + (n - 1) * C: _C1S_OFF + n * C]
                c2v = tab_t[:, _C2S_OFF + (n - 1) * C: _C2S_OFF + n * C]
                t1 = spool.tile([BPC, C], F32, tag="lag1")
                nc.vector.tensor_scalar(t1[:], c1v, xs[:, n - 1: n], None,
                                        OP.subtract)
                t2 = spool.tile([BPC, C], F32, tag="lag2")
                nc.vector.tensor_tensor(t2[:], t1[:], cur, OP.mult)
                t3 = spool.tile([BPC, C], F32, tag="lag3")
                nc.vector.tensor_tensor(t3[:], c2v, prev, OP.mult)
                nc.vector.tensor_tensor(nxt, t2[:], t3[:], OP.subtract)

            # ---- assemble Dt[b, n*32+m] = D_b[m, n] (without e^{-x/2}) ----
            # upper part m = n+d (d >= 0): diag positions n*33 + d, n = 0..31-d
            #   value = fd[d][n] * Lag[n, d] * alpha^d
            # lower part n = m+d (d >= 1): diag positions m*33 + 32*d, m = 0..31-d
            #   value = fd[d][m] * Lag[m, d] * beta^d
            for d in range(C):
                w = C - d
                lagv = Lag[:, d: d + (w - 1) * C + 1: C]          # (BPC, w)
                fdv = tab_t[:, _FD_OFFS[d]: _FD_OFFS[d] + w]
                t = spool.tile([BPC, C], F32, tag="asm")
                nc.vector.tensor_tensor(t[:, :w], lagv, fdv, OP.mult)
                up = slice(d, d + (w - 1) * 33 + 1, 33)
                nc.vector.tensor_scalar_mul(
                    Dt_re[:, up], t[:, :w], PaRe[:, d: d + 1])
                nc.vector.tensor_scalar_mul(
                    Dt_im[:, up], t[:, :w], PaIm[:, d: d + 1])
                if d >= 1:
                    lo = slice(32 * d, 32 * d + (w - 1) * 33 + 1, 33)
                    nc.vector.tensor_scalar_mul(
                        Dt_re[:, lo], t[:, :w], PbRe[:, d: d + 1])
                    nc.vector.tensor_scalar_mul(
                        Dt_im[:, lo], t[:, :w], PbIm[:, d: d + 1])
            # fold e^{-x/2}
            nc.vector.tensor_scalar_mul(Dt_re[:], Dt_re[:], e2[:])
            nc.vector.tensor_scalar_mul(Dt_im[:], Dt_im[:], e2[:])

            # ---- scatter D^T blocks onto the block-diagonal lhsT ----
            # lhsT[b*32+n, b*32+m] = D_b[m, n] = Dt[b, n*32+m]
            # (memset can't write f32r; zero via DMA from a zeroed f32 tile)
            zscr = cpool.tile([P, P], F32)
            nc.vector.memset(zscr[:], 0.0)
            nc.sync.dma_start(out=lhsT_re[:], in_=zscr[:].bitcast(MMDT))
            nc.sync.dma_start(out=lhsT_im[:], in_=zscr[:].bitcast(MMDT))
            for b in range(BPC):
                nc.sync.dma_start(
                    out=lhsT_re[C * b: C * (b + 1), C * b: C * (b + 1)],
                    in_=Dt_re[b: b + 1, :].bitcast(MMDT),
                )
                nc.sync.dma_start(
                    out=lhsT_im[C * b: C * (b + 1), C * b: C * (b + 1)],
                    in_=Dt_im[b: b + 1, :].bitcast(MMDT),
                )

            # ---- main loop: 64 chunks of 512 state columns ----
            # state DMA'd in 2MB (128 x 4096 f32) pieces, output written in
            # 2MB pieces (4 chunks x 1024 interleaved f32).
            out_eng = nc.scalar if out_on_act else nc.sync
            # st_f32: state-piece width (f32 cols) per in-DMA
            # ob_f32: out-piece width (f32, interleaved) per out-DMA
            #         = ob_f32 // 1024 matmul subtiles of 512 state cols
            n_grp = 2 * NFREE // ob_f32
            sub_per_ob = ob_f32 // 1024
            sub_per_st = st_f32 // 512
            for _ in range(niter):
                schunk = None
                for g in range(n_grp):
                    obuf = opool.tile([P, ob_f32], F32, tag="ob")
                    psr, psi = [], []
                    for i in range(sub_per_ob):
                        q = g * sub_per_ob + i          # global 512-col index
                        if q % sub_per_st == 0:
                            schunk = stpool.tile([P, st_f32], MMDT, tag="st")
                            nc.sync.dma_start(
                                out=schunk[:],
                                in_=state[:, q * 512: q * 512 + st_f32]
                                .bitcast(MMDT),
                            )
                        lo = (q % sub_per_st) * 512
                        ps = ppool.tile([P, 512], F32, tag="ps")
                        nc.tensor.matmul(
                            ps[:], lhsT_re[:], schunk[:, lo: lo + 512],
                            start=True, stop=True,
                        )
                        psr.append(ps)
                        ps = ppool.tile([P, 512], F32, tag="ps")
                        nc.tensor.matmul(
                            ps[:], lhsT_im[:], schunk[:, lo: lo + 512],
                            start=True, stop=True,
                        )
                        psi.append(ps)
                    for i in range(sub_per_ob):
                        nc.vector.tensor_copy(
                            obuf[:, i * 1024: (i + 1) * 1024: 2], psr[i][:]
                        )
                        nc.scalar.copy(
                            obuf[:, i * 1024 + 1: (i + 1) * 1024: 2], psi[i][:]
                        )
                    out_eng.dma_start(
                        out=out[:, g * ob_f32: (g + 1) * ob_f32], in_=obuf[:]
                    )

    nc.compile()
    return nc


_CACHE: dict = {}


def _get_nc(niter: int = 1):
    if niter not in _CACHE:
        _CACHE[niter] = build_nc(niter)
    return _CACHE[niter]


def kernel(r: np.ndarray, phi: np.ndarray, state: np.ndarray) -> np.ndarray:
    from concourse.bass_utils import run_bass_kernel_spmd

    r = np.ascontiguousarray(r, dtype=np.float32)
    phi = np.ascontiguousarray(phi, dtype=np.float32)
    state = np.ascontiguousarray(state, dtype=np.float32)
    assert r.shape == (B,) and phi.shape == (B,)
    assert state.shape == (B, C, C, C, C)

    nc = _get_nc(1)
    tab = _build_tables()
    in_maps = []
    for c in range(NCORES):
        b0 = c * BPC
        in_maps.append({
            "state": state[b0: b0 + BPC].reshape(P, NFREE),
            "rphi": np.stack(
                [r[b0: b0 + BPC], phi[b0: b0 + BPC]], axis=1
            ).astype(np.float32),
            "tab": tab,
        })

    res = run_bass_kernel_spmd(nc, in_maps, core_ids=list(range(NCORES)))

    outp = np.empty((B, C, C, C, C), dtype=np.complex64)
    for c in range(NCORES):
        b0 = c * BPC
        blk = res.results[c]["out"]
        outp[b0: b0 + BPC] = (
            np.ascontiguousarray(blk).view(np.complex64).reshape(BPC, C, C, C, C)
        )
    return outp
